# revision 12
# baseline (speedup 1.0000x reference)
"""2-layer GCN (GCNConv -> ReLU -> GCNConv -> edge dot products) on 8 TRN2
NeuronCores via Bass/Tile.

Math: with A' = A + I (self loops), deg = in-degree of A', dinv = deg^-1/2:
    h  = relu(dinv_d * sum_{e->d} [dinv_s * (x_s @ W1)] + b1)
    z  = dinv_d * sum_{e->d} [dinv_s * (h_s @ W2)] + b2
    out[k] = dot(z[src_k], z[dst_k])  over edge_label_index
The symmetric norm factors out of the edge sum: we scale table rows by dinv
before the gather and scale the aggregated result by dinv after.

Sharding: nodes are range-sharded over 8 cores (12500 each); edges are
partitioned by destination core, sorted by (dst tile, src window) on the
host.  The gather table is replicated via AllGather, split in two halves
(window A = first 6272 rows of each shard, window B = the rest) so each
half's collective can fire as soon as the producing tiles are done and
overlap with the remaining tiles' compute.  Each core:
  stage0: xw1' = dinv * (x_shard @ W1)    -> AG-A after tile 48, AG-B after 97
  L1:     per 128-dst tile, gather xw1'[src] rows (dma_gather), build the
          tile's one-hot P blocks with ONE batched iota==dst compare,
          accumulate P^T @ G in PSUM, post-scale + bias + relu -> h;
          hw2' = dinv * (h @ W2) -> chunked AllGather as tiles complete
  L2:     same aggregation over hw2' -> z shard -> chunked AllGather (f32)
  final:  gather z[src], z[dst] for its 25k label pairs (AA group first so
          it can start as soon as the A-half table arrived), mul + reduce.

dma_gather constraints shape the layout: int16 indices (signed offsets from a
window-table base), <=1024 indices per call (HW ring), and trailing-negative
indices are trimmed -- so partition 127 of every 128-slot block is a pad
(idx 0), making each call's last slot non-negative.  Host-side work is index
manipulation only (bucketing/sorting/padding) plus the degree histogram that
falls out of the dst partition; all floating-point math runs on device.
"""
import os
import sys

sys.path.insert(0, "/opt/trn_rl_repo")

import numpy as np
import ml_dtypes

# debug bisection: 0=stage0+AG1, 1=+L1+AG2, 2=+L2+AG3, 3=full (default)
PHASE = int(os.environ.get("GCN_PHASE", "3"))
# replace collectives with local DMA copies (single-core timeline analysis)
NOCC = bool(int(os.environ.get("GCN_NOCC", "0")))
# ablations for differential timing: "", "nogather"
ABLATE = os.environ.get("GCN_ABLATE", "")
# dma_gather single_packet flag A/B
SINGLE_PACKET = bool(int(os.environ.get("GCN_SP", "1")))

import concourse.bacc as bacc
import concourse.bass as bass
import concourse.mybir as mybir
import concourse.tile as tile
from concourse.bass_utils import run_bass_kernel_spmd

F32 = mybir.dt.float32
BF16 = mybir.dt.bfloat16
I16 = mybir.dt.int16

N = 100000
NCORES = 8
NS = N // NCORES            # 12500 nodes per core
T = (NS + 127) // 128       # 98 dst tiles per core
NP = T * 128                # padded shard nodes (12544)
LAST_ROWS = NS - 128 * (T - 1)   # 84 valid rows in the last tile
C_IN = 256
HID = 128
OUT = 64
NW = 2                      # index windows = AllGather chunks (A/B)
# Window A is deliberately the SMALLER chunk (~3/8): its AllGather fires
# early and the w0 gather work it unlocks keeps GpSimd busy while the
# B-half collective is still in flight.  B must stay under 65536 table
# rows so signed-int16 offsets reach it from the window midpoint.
TS = int(os.environ.get("GCN_TS", "37"))  # tiles per core in window A
SPLIT_OFF = TS * 128
NA_PER = SPLIT_OFF          # window-A rows per core
NB_PER = NS - SPLIT_OFF     # window-B rows per core
NA = NA_PER * NCORES        # window-A table rows
NBT = NB_PER * NCORES       # window-B table rows
assert NA < 65536 and NBT < 65536
WBASE = (NA // 2, NBT // 2)      # signed-int16 index bases per window
BT = 7                      # dst tiles per gather batch
NBATCH = T // BT            # 14
E_LBL = 200000
LS = E_LBL // NCORES        # 25000 label pairs per core
EPB = 127                   # edges per 128-slot block (slot 127 = pad)
CALL_BLOCKS = int(os.environ.get("GCN_CB", "8"))  # blocks per dma_gather call
LBL_CB = 8                  # blocks per label-gather piece (bounds zs/zd size)
SGRP = 14                   # stage0 tiles loaded per DMA

# exported for test harness introspection
LAST_RESULTS = None

_PROGRAM_CACHE = {}


# --------------------------------------------------------------- static layout

def _layout(cfg):
    """All static offsets derived from cfg = (nbw, lg).

    nbw: per-tile (nb_w0, nb_w1) block budgets.  lg: 4 label-group budgets.
    """
    nbw, lg = cfg
    tb_off = []                 # dstloc/matmul column base per tile
    s = 0
    for t in range(T):
        tb_off.append(s)
        s += nbw[t][0] + nbw[t][1]
    TB = s                      # total blocks per layer pass

    batches = []
    ecol = 0                    # running eidx int16 column offset
    for b in range(NBATCH):
        tiles = list(range(b * BT, (b + 1) * BT))
        reg = [sum(nbw[t][w] for t in tiles) for w in range(NW)]
        gcol = {}               # (tile, w) -> G column base for this batch
        for w in range(NW):
            base = 0 if w == 0 else reg[0]
            for t in tiles:
                gcol[(t, w)] = base
                base += nbw[t][w]
        calls = []              # (w, g_col, nblocks, ecol_off)
        for w in range(NW):
            off = 0
            while off < reg[w]:
                nb = min(reg[w] - off, CALL_BLOCKS)
                calls.append((w, (0 if w == 0 else reg[0]) + off, nb, ecol))
                ecol += nb * 8
                off += nb
        batches.append({"tiles": tiles, "reg": reg, "gcol": gcol,
                        "calls": calls})
    ecols = ecol

    lgoff = [0]
    for v in lg:
        lgoff.append(lgoff[-1] + v)
    lblk = lgoff[-1]
    pieces = []                 # (w1, w2, block_off, nblocks)
    for g in range(NW * NW):
        nb = lg[g]
        off = lgoff[g]
        while nb > 0:
            take = min(nb, LBL_CB)
            pieces.append((g // NW, g % NW, off, take))
            off += take
            nb -= take
    maxnb = max(nbw[t][0] + nbw[t][1] for t in range(T))
    return {"tb_off": tb_off, "TB": TB, "batches": batches, "ecols": ecols,
            "lgoff": lgoff, "lblk": lblk, "pieces": pieces, "maxnb": maxnb}


# ----------------------------------------------------------------- host prep

def _win_of(v):
    """Window (AllGather chunk) and table row of global node id v."""
    c, off = v // NS, v % NS
    w = (off >= SPLIT_OFF).astype(np.int64)
    row = np.where(w == 0, c * NA_PER + off, c * NB_PER + (off - SPLIT_OFF))
    return w, row


def _pack_idx(flat):
    """dma_gather index layout: arr[j, i] = flat[i*16 + j], tiled to 128."""
    arr = np.asarray(flat, dtype=np.int16).reshape(-1, 16).T
    return np.tile(arr, (8, 1))


def _fill_blocks(flat, base_slot, values):
    """Place `values` into 128-slot blocks at flat[base_slot:], 127 per block
    (slot 127 reserved as pad)."""
    i = np.arange(len(values))
    pos = base_slot + (i // EPB) * 128 + (i % EPB)
    flat[pos] = values


def _prep(x, edge_index, edge_label_index, W1, b1, W2, b2):
    src = np.asarray(edge_index[0], dtype=np.int64)
    dst = np.asarray(edge_index[1], dtype=np.int64)

    # degree histogram includes the implicit self-loops; the self-loop
    # edges themselves are NOT gathered -- their contribution is added from
    # the locally-computed table rows in the per-tile post ops.
    deg = (np.bincount(dst, minlength=N) + 1).astype(np.float32)

    core_of = dst // NS

    w_all, row_all = _win_of(src)

    per_core = []
    cnts = np.zeros((NCORES, T, NW), np.int64)
    for c in range(NCORES):
        m = core_of == c
        r = row_all[m]
        dl = dst[m] - c * NS
        tl = dl >> 7
        loc = (dl & 127).astype(np.float32)
        w = w_all[m]
        order = np.lexsort((r, w, tl))   # by tile, window, then row (locality)
        r, tl, loc, w = r[order], tl[order], loc[order], w[order]
        cnt = np.bincount(tl * NW + w, minlength=T * NW).reshape(T, NW)
        cnts[c] = cnt
        per_core.append((r, loc, cnt))

    nbw = tuple(
        tuple(int(v) for v in
              np.ceil(cnts[:, t, :].max(axis=0) / EPB).astype(np.int64))
        for t in range(T))

    # label pairs: shard by index, double-bucket by (src window, dst window)
    lsrc = np.asarray(edge_label_index[0], dtype=np.int64)
    ldst = np.asarray(edge_label_index[1], dtype=np.int64)
    lw1_all, lr1_all = _win_of(lsrc)
    lw2_all, lr2_all = _win_of(ldst)
    lab_core = []
    lcnts = np.zeros((NCORES, NW * NW), np.int64)
    for c in range(NCORES):
        sl = slice(c * LS, (c + 1) * LS)
        r1, r2 = lr1_all[sl], lr2_all[sl]
        w1_, w2_ = lw1_all[sl], lw2_all[sl]
        order = np.lexsort((w2_, w1_))
        g = (w1_ * NW + w2_)[order]
        lcnts[c] = np.bincount(g, minlength=NW * NW)
        lab_core.append((r1[order], r2[order], order))
    lg = tuple(int(v) for v in
               np.ceil(lcnts.max(axis=0) / EPB).astype(np.int64))

    cfg = (nbw, lg)
    lay = _layout(cfg)

    iota_w = np.broadcast_to(
        np.arange(128, dtype=np.float32),
        (128, lay["maxnb"], 128)).reshape(128, -1).astype(ml_dtypes.bfloat16)
    ident = np.eye(128, dtype=np.float32).astype(ml_dtypes.bfloat16)
    w1m = np.asarray(W1, np.float32).astype(ml_dtypes.bfloat16)      # [256,128]
    w2p = np.zeros((HID, 128), np.float32)
    w2p[:, :OUT] = np.asarray(W2, np.float32)
    w2p = w2p.astype(ml_dtypes.bfloat16)
    b1r = np.broadcast_to(np.asarray(b1, np.float32), (128, HID)).copy()
    b2r = np.broadcast_to(np.asarray(b2, np.float32), (128, OUT)).copy()

    xf = np.asarray(x, np.float32)

    in_maps = []
    slot2orig = []
    for c in range(NCORES):
        r, loc, cnt = per_core[c]
        gstart = np.concatenate([[0], np.cumsum(cnt.reshape(-1))]).astype(np.int64)

        eflat = np.zeros(lay["ecols"] * 16, np.int16)
        dlflat = np.full(lay["TB"] * 128, 255.0, np.float32)
        for b in range(NBATCH):
            bi = lay["batches"][b]
            # eidx slot base of this batch's stream = 16 * ecol of first call
            sbase = bi["calls"][0][3] * 16
            for w in range(NW):
                for t in bi["tiles"]:
                    n_e = int(cnt[t, w])
                    if not n_e:
                        continue
                    gi = t * NW + w
                    vals = (r[gstart[gi]:gstart[gi] + n_e]
                            - WBASE[w]).astype(np.int16)
                    _fill_blocks(eflat, sbase + bi["gcol"][(t, w)] * 128, vals)
                    dcol = lay["tb_off"][t] + (nbw[t][0] if w else 0)
                    _fill_blocks(dlflat, dcol * 128,
                                 loc[gstart[gi]:gstart[gi] + n_e])
        eidx = _pack_idx(eflat)
        dstloc = dlflat.reshape(lay["TB"], 128).T.astype(ml_dtypes.bfloat16)

        # label indices
        r1, r2, order = lab_core[c]
        lcnt = lcnts[c]
        lblk = lay["lblk"]
        lsflat = np.zeros(lblk * 128, np.int16)
        ldflat = np.zeros(lblk * 128, np.int16)
        s2o = np.full(lblk * 128, -1, np.int64)
        pos = 0
        for g in range(NW * NW):
            n_p = int(lcnt[g])
            base = lay["lgoff"][g] * 128
            if n_p:
                _fill_blocks(lsflat, base,
                             (r1[pos:pos + n_p] - WBASE[g // NW]).astype(np.int16))
                _fill_blocks(ldflat, base,
                             (r2[pos:pos + n_p] - WBASE[g % NW]).astype(np.int16))
                _fill_blocks(s2o, base, order[pos:pos + n_p])
            pos += n_p
        slot2orig.append(s2o)

        xs = xf[c * NS:(c + 1) * NS]
        xT = np.zeros((C_IN, NP), np.float32)
        xT[:, :NS] = xs.T
        degc = np.ones(NP, np.float32)
        degc[:NS] = deg[c * NS:(c + 1) * NS]

        in_maps.append({
            "xT": xT.astype(ml_dtypes.bfloat16),
            "w1": w1m, "w2p": w2p, "b1r": b1r, "b2r": b2r,
            "iota_w": iota_w, "ident": ident,
            "deg": degc.reshape(T, 128).T.copy(),
            "dstloc": dstloc,
            "eidx": eidx,
            "lsidx": _pack_idx(lsflat),
            "ldidx": _pack_idx(ldflat),
        })
    return cfg, in_maps, slot2orig


# ------------------------------------------------------------- device program

def _build(cfg):
    nbw, lg = cfg
    lay = _layout(cfg)
    TB = lay["TB"]
    ecols = lay["ecols"]
    maxnb = lay["maxnb"]
    lblk = lay["lblk"]
    lcols = lblk * 8
    gw = max(sum(bi["reg"]) for bi in lay["batches"])   # max G width (blocks)

    nc = bacc.Bacc("TRN2", target_bir_lowering=False, debug=False,
                   num_devices=1 if NOCC else NCORES, num_swdge_queues=4)

    xT_d = nc.dram_tensor("xT", [C_IN, NP], BF16, kind="ExternalInput")
    w1_d = nc.dram_tensor("w1", [C_IN, HID], BF16, kind="ExternalInput")
    w2p_d = nc.dram_tensor("w2p", [HID, 128], BF16, kind="ExternalInput")
    b1r_d = nc.dram_tensor("b1r", [128, HID], F32, kind="ExternalInput")
    b2r_d = nc.dram_tensor("b2r", [128, OUT], F32, kind="ExternalInput")
    iota_w_d = nc.dram_tensor("iota_w", [128, maxnb * 128], BF16,
                              kind="ExternalInput")
    ident_d = nc.dram_tensor("ident", [128, 128], BF16, kind="ExternalInput")
    deg_d = nc.dram_tensor("deg", [128, T], F32, kind="ExternalInput")
    dstloc_d = nc.dram_tensor("dstloc", [128, TB], BF16, kind="ExternalInput")
    eidx_d = nc.dram_tensor("eidx", [128, ecols], I16, kind="ExternalInput")
    lsidx_d = nc.dram_tensor("lsidx", [128, lcols], I16, kind="ExternalInput")
    ldidx_d = nc.dram_tensor("ldidx", [128, lcols], I16, kind="ExternalInput")
    out_d = nc.dram_tensor("out_lbl", [128, lblk], F32, kind="ExternalOutput")
    if PHASE == 0:
        dbgA_d = nc.dram_tensor("dbgA", [NA, HID], BF16, kind="ExternalOutput")
        dbgB_d = nc.dram_tensor("dbgB", [NBT, HID], BF16, kind="ExternalOutput")
    elif PHASE == 1:
        dbgA_d = nc.dram_tensor("dbgA", [NA, 128], BF16, kind="ExternalOutput")
        dbgB_d = nc.dram_tensor("dbgB", [NBT, 128], BF16, kind="ExternalOutput")
    elif PHASE == 2:
        dbgA_d = nc.dram_tensor("dbgA", [NA, OUT], F32, kind="ExternalOutput")
        dbgB_d = nc.dram_tensor("dbgB", [NBT, OUT], F32, kind="ExternalOutput")

    # per-layer (A, B) collective buffers
    cc_in = []
    cc_out = []
    for li, (ch, dt) in enumerate([(HID, BF16), (128, BF16), (OUT, F32)]):
        ai = nc.dram_tensor(f"cc{li}a_in", [NA_PER, ch], dt)
        bi_ = nc.dram_tensor(f"cc{li}b_in", [NB_PER, ch], dt)
        ao = nc.dram_tensor(f"cc{li}a_out", [NA, ch], dt, addr_space="Shared")
        bo = nc.dram_tensor(f"cc{li}b_out", [NBT, ch], dt, addr_space="Shared")
        cc_in.append((ai, bi_))
        cc_out.append((ao, bo))

    rg = [list(range(NCORES))]
    mult = mybir.AluOpType.mult
    add = mybir.AluOpType.add
    iseq = mybir.AluOpType.is_equal
    Relu = mybir.ActivationFunctionType.Relu

    def emit_ag(li, w):
        src_t, dst_t = cc_in[li][w], cc_out[li][w]
        if NOCC:
            n = NA_PER if w == 0 else NB_PER
            nc.sync.dma_start(dst_t[0:n, :], src_t[:])
        else:
            nc.gpsimd.collective_compute(
                "AllGather", mybir.AluOpType.bypass, replica_groups=rg,
                ins=[src_t[:]], outs=[dst_t[:]])

    with tile.TileContext(nc) as tc:
        with tc.tile_pool(name="const", bufs=1) as cpool, \
             tc.tile_pool(name="work", bufs=3) as wpool, \
             tc.tile_pool(name="gbuf", bufs=2) as gpool, \
             tc.tile_pool(name="pbuf", bufs=3) as ppool, \
             tc.tile_pool(name="psum", bufs=2, space="PSUM") as pspool:

            # ---- constants
            iota_w_sb = cpool.tile([128, maxnb, 128], BF16)
            nc.sync.dma_start(
                iota_w_sb[:],
                iota_w_d[:].rearrange("p (b j) -> p b j", b=maxnb))
            ident_sb = cpool.tile([128, 128], BF16)
            nc.sync.dma_start(ident_sb[:], ident_d[:])
            b1r_sb = cpool.tile([128, HID], F32)
            nc.sync.dma_start(b1r_sb[:], b1r_d[:])
            b2r_sb = cpool.tile([128, OUT], F32)
            nc.sync.dma_start(b2r_sb[:], b2r_d[:])
            w1_sb = cpool.tile([128, 2, HID], BF16)
            nc.sync.dma_start(w1_sb[:, 0, :], w1_d[0:128, :])
            nc.sync.dma_start(w1_sb[:, 1, :], w1_d[128:256, :])
            w2p_sb = cpool.tile([128, 128], BF16)
            nc.sync.dma_start(w2p_sb[:], w2p_d[:])
            dstloc_sb = cpool.tile([128, TB], BF16)
            nc.sync.dma_start(dstloc_sb[:], dstloc_d[:])
            eidx_sb = cpool.tile([128, ecols], I16)
            nc.sync.dma_start(eidx_sb[:], eidx_d[:])
            lsidx_sb = cpool.tile([128, lcols], I16)
            nc.sync.dma_start(lsidx_sb[:], lsidx_d[:])
            ldidx_sb = cpool.tile([128, lcols], I16)
            nc.sync.dma_start(ldidx_sb[:], ldidx_d[:])

            deg_sb = cpool.tile([128, T], F32)
            nc.sync.dma_start(deg_sb[:], deg_d[:])
            rec_sb = cpool.tile([128, T], F32)
            nc.vector.reciprocal(rec_sb[:], deg_sb[:])
            dinv = cpool.tile([128, T], F32)
            nc.scalar.sqrt(dinv[:], rec_sb[:])

            qctr = [0]

            def stage0_store(t, tb, rows):
                if t < TS:
                    nc.sync.dma_start(
                        cc_in[0][0][t * 128:t * 128 + rows, :], tb[:rows, :])
                else:
                    r0 = (t - TS) * 128
                    nc.sync.dma_start(
                        cc_in[0][1][r0:r0 + rows, :], tb[:rows, :])

            def emit_body():
                # ---- stage 0: xw1' = dinv * (x @ W1), bf16 table shard
                for grp in range(T // SGRP):
                    xt = wpool.tile([128, 2, SGRP * 128], BF16, tag="xt")
                    c0 = grp * SGRP * 128
                    nc.sync.dma_start(xt[:, 0, :],
                                      xT_d[0:128, c0:c0 + SGRP * 128])
                    nc.sync.dma_start(xt[:, 1, :],
                                      xT_d[128:256, c0:c0 + SGRP * 128])
                    for j in range(SGRP):
                        t = grp * SGRP + j
                        sl = slice(j * 128, (j + 1) * 128)
                        ps = pspool.tile([128, HID], F32, tag="ps0")
                        nc.tensor.matmul(ps[:], lhsT=xt[:, 0, sl],
                                         rhs=w1_sb[:, 0, :],
                                         start=True, stop=False)
                        nc.tensor.matmul(ps[:], lhsT=xt[:, 1, sl],
                                         rhs=w1_sb[:, 1, :],
                                         start=False, stop=True)
                        tb = wpool.tile([128, HID], BF16, tag="tb0")
                        nc.vector.tensor_scalar(out=tb[:], in0=ps[:],
                                                scalar1=dinv[:, t:t + 1],
                                                scalar2=None, op0=mult)
                        rows = 128 if t < T - 1 else LAST_ROWS
                        stage0_store(t, tb, rows)
                    if grp == (TS - 1) // SGRP:
                        emit_ag(0, 0)
                emit_ag(0, 1)

                def agg_layer(li, n_ch, post_fn, width):
                    """Aggregate over edges reading cc_out[li], write via
                    post_fn; fire the next layer's chunked AG as the A/B tile
                    groups complete (caller does that via post_fn hooks)."""
                    tables = cc_out[li]
                    for bnum, bi in enumerate(lay["batches"]):
                        g = gpool.tile([128, gw, n_ch], BF16, tag="G")
                        for (w, g_col, nb, ecol) in bi["calls"]:
                            if ABLATE == "nogather":
                                continue
                            nidx = nb * 128
                            nc.gpsimd.dma_gather(
                                g[:, g_col:g_col + nb, :],
                                tables[w][WBASE[w]:, :],
                                eidx_sb[:, ecol:ecol + nidx // 16],
                                nidx, nidx, n_ch,
                                single_packet=SINGLE_PACKET,
                                queue_num=qctr[0] % 4)
                            qctr[0] += 1
                        for t in bi["tiles"]:
                            nbt_t = nbw[t][0] + nbw[t][1]
                            p_all = ppool.tile([128, maxnb, 128], BF16,
                                               tag="P")
                            c0 = lay["tb_off"][t]
                            nc.vector.tensor_tensor(
                                out=p_all[:, 0:nbt_t, :],
                                in0=iota_w_sb[:, 0:nbt_t, :],
                                in1=dstloc_sb[:, c0:c0 + nbt_t]
                                    .unsqueeze(2)
                                    .to_broadcast([128, nbt_t, 128]),
                                op=iseq)
                            ps = pspool.tile([128, width], F32, tag="agg")
                            k = 0
                            for w in range(NW):
                                for j in range(nbw[t][w]):
                                    col = bi["gcol"][(t, w)] + j
                                    nc.tensor.matmul(
                                        ps[:], lhsT=p_all[:, k, :],
                                        rhs=g[:, col, :width],
                                        start=(k == 0), stop=(k == nbt_t - 1))
                                    k += 1
                            post_fn(t, ps)

                def self_rows(li, t, rows, ch, tag):
                    """Load this tile's own table rows (the self-loop term)."""
                    sb = wpool.tile([128, ch], BF16, tag=tag)
                    if t < TS:
                        nc.sync.dma_start(
                            sb[:rows, :],
                            cc_in[li][0][t * 128:t * 128 + rows, :])
                    else:
                        r0 = (t - TS) * 128
                        nc.sync.dma_start(
                            sb[:rows, :], cc_in[li][1][r0:r0 + rows, :])
                    return sb

                def mk_post_l1():
                    def post_l1(t, ps):
                        rows = 128 if t < T - 1 else LAST_ROWS
                        # table row already carries one dinv factor; the
                        # outer dinv scale below supplies the second.
                        selfb = self_rows(0, t, rows, HID, "self1")
                        tmp0 = wpool.tile([128, HID], F32, tag="tmp0")
                        nc.vector.tensor_tensor(
                            out=tmp0[:], in0=ps[:], in1=selfb[:], op=add)
                        tmp = wpool.tile([128, HID], F32, tag="tmp1")
                        nc.vector.scalar_tensor_tensor(
                            out=tmp[:], in0=tmp0[:], scalar=dinv[:, t:t + 1],
                            in1=b1r_sb[:], op0=mult, op1=add)
                        hsb = wpool.tile([128, HID], BF16, tag="hsb")
                        nc.scalar.activation(hsb[:], tmp[:], Relu)
                        psT = pspool.tile([128, 128], BF16, tag="psT")
                        nc.tensor.transpose(psT[:], hsb[:], ident_sb[:])
                        hT = wpool.tile([128, 128], BF16, tag="hT")
                        nc.vector.tensor_copy(hT[:], psT[:])
                        ps2 = pspool.tile([128, 128], F32, tag="hw2")
                        nc.tensor.matmul(ps2[:], lhsT=hT[:], rhs=w2p_sb[:],
                                         start=True, stop=True)
                        h2 = wpool.tile([128, 128], BF16, tag="h2")
                        nc.vector.tensor_scalar(out=h2[:], in0=ps2[:],
                                                scalar1=dinv[:, t:t + 1],
                                                scalar2=None, op0=mult)
                        if t < TS:
                            nc.sync.dma_start(
                                cc_in[1][0][t * 128:t * 128 + rows, :],
                                h2[:rows, :])
                        else:
                            r0 = (t - TS) * 128
                            nc.sync.dma_start(
                                cc_in[1][1][r0:r0 + rows, :], h2[:rows, :])
                        if t == TS - 1:
                            emit_ag(1, 0)
                        elif t == T - 1:
                            emit_ag(1, 1)
                    return post_l1

                def mk_post_l2():
                    def post_l2(t, ps):
                        rows = 128 if t < T - 1 else LAST_ROWS
                        selfb = self_rows(1, t, rows, 128, "self2")
                        tmp0 = wpool.tile([128, OUT], F32, tag="tmp2")
                        nc.vector.tensor_tensor(
                            out=tmp0[:], in0=ps[:], in1=selfb[:, 0:OUT],
                            op=add)
                        z = wpool.tile([128, OUT], F32, tag="z")
                        nc.vector.scalar_tensor_tensor(
                            out=z[:], in0=tmp0[:], scalar=dinv[:, t:t + 1],
                            in1=b2r_sb[:], op0=mult, op1=add)
                        if t < TS:
                            nc.sync.dma_start(
                                cc_in[2][0][t * 128:t * 128 + rows, :],
                                z[:rows, :])
                        else:
                            r0 = (t - TS) * 128
                            nc.sync.dma_start(
                                cc_in[2][1][r0:r0 + rows, :], z[:rows, :])
                        if t == TS - 1:
                            emit_ag(2, 0)
                        elif t == T - 1:
                            emit_ag(2, 1)
                    return post_l2

                if PHASE == 0:
                    nc.sync.dma_start(dbgA_d[:], cc_out[0][0][:])
                    nc.sync.dma_start(dbgB_d[:], cc_out[0][1][:])

                if PHASE >= 1:
                    agg_layer(0, HID, mk_post_l1(), HID)
                if PHASE == 1:
                    nc.sync.dma_start(dbgA_d[:], cc_out[1][0][:])
                    nc.sync.dma_start(dbgB_d[:], cc_out[1][1][:])

                if PHASE >= 2:
                    agg_layer(1, 128, mk_post_l2(), OUT)
                if PHASE == 2:
                    nc.sync.dma_start(dbgA_d[:], cc_out[2][0][:])
                    nc.sync.dma_start(dbgB_d[:], cc_out[2][1][:])

                if PHASE >= 3:
                    # ---- final: label-edge dot products (AA group first so
                    # it only waits on the A-half AllGather)
                    out_sb = cpool.tile([128, lblk], F32, tag="out_sb")
                    for (w1_, w2_, po, nb) in lay["pieces"]:
                        nidx = nb * 128
                        zs = wpool.tile([128, LBL_CB, OUT], F32, tag="zs")
                        nc.gpsimd.dma_gather(
                            zs[:, 0:nb, :], cc_out[2][w1_][WBASE[w1_]:, :],
                            lsidx_sb[:, po * 8:po * 8 + nidx // 16],
                            nidx, nidx, OUT,
                            single_packet=SINGLE_PACKET,
                            queue_num=qctr[0] % 4)
                        qctr[0] += 1
                        zd = wpool.tile([128, LBL_CB, OUT], F32, tag="zd")
                        nc.gpsimd.dma_gather(
                            zd[:, 0:nb, :], cc_out[2][w2_][WBASE[w2_]:, :],
                            ldidx_sb[:, po * 8:po * 8 + nidx // 16],
                            nidx, nidx, OUT,
                            single_packet=SINGLE_PACKET,
                            queue_num=qctr[0] % 4)
                        qctr[0] += 1
                        pr = wpool.tile([128, LBL_CB, OUT], F32, tag="pr")
                        nc.vector.tensor_tensor(out=pr[:, 0:nb, :],
                                                in0=zs[:, 0:nb, :],
                                                in1=zd[:, 0:nb, :], op=mult)
                        nc.vector.tensor_reduce(out=out_sb[:, po:po + nb],
                                                in_=pr[:, 0:nb, :],
                                                axis=mybir.AxisListType.X,
                                                op=add)
                    nc.sync.dma_start(out_d[:], out_sb[:])

            emit_body()

    nc.compile()
    return nc


def _get_program(cfg):
    if cfg not in _PROGRAM_CACHE:
        _PROGRAM_CACHE[cfg] = _build(cfg)
    return _PROGRAM_CACHE[cfg]


# ------------------------------------------------------------------ entrypoint

def kernel(x, edge_index, edge_label_index, W1, b1, W2, b2):
    global LAST_RESULTS
    cfg, in_maps, slot2orig = _prep(x, edge_index, edge_label_index,
                                    W1, b1, W2, b2)
    nc = _get_program(cfg)
    res = run_bass_kernel_spmd(nc, in_maps, core_ids=list(range(NCORES)))
    LAST_RESULTS = res
    out = np.empty(E_LBL, np.float32)
    for c in range(NCORES):
        vals = res.results[c]["out_lbl"].T.reshape(-1)   # slot-ordered
        s2o = slot2orig[c]
        valid = s2o >= 0
        out[c * LS + s2o[valid]] = vals[valid]
    return out
